# revision 1
# baseline (speedup 1.0000x reference)
"""Trainium2 Bass kernel for nn_MultiHeadAttention_3796751090171 (sparse_attention).

Batch-parallel SPMD across 8 NeuronCores: q_batch/k_batch are SORTED, so the
cross-batch mask makes attention block-diagonal over batches, and there are
exactly B=8 batches for 8 cores. Core c computes batch c's queries against
batch c's keys for ALL 8 heads -- completely independent work, so there are
NO collectives: the full output is a pure row-concatenation of the per-core
outputs.

Uniform SPMD template: every core runs the same program on [NKMAX x NQMAX]
padded tiles (NQMAX/NKMAX = max batch size rounded up to 128). The host pads
each core's feature slices with zeros and fabricates the pos_enc tile so that
  - padded k-rows carry pos = -1e9  -> exp = 0, no contribution to h or Z
  - padded q-cols carry pos = 0 on real k-rows (keeps Z finite; outputs for
    those columns are dropped by the host)

Per core c (batch slice qs:qe / ks:ke, all heads h):
  Q = qf[qs:qe] @ Wq/8, K = kf[ks:ke] @ Wk, V = vf[ks:ke] @ Wv  (+biases)
  per head: scoresT[k,q] = K_h^T-chunks @ Q_h + posT  (PSUM)
  expT = exp(scoresT); hT_unnorm/Z via [V|1] matmul (ones column -> row 64 = Z)
  hTn = hT * (1/Z broadcast); out^T[o,q] = sum_h Wo[64h:,:].T @ hTn_h + bo
Host: out[qs:qe, :] = outT[:, :nq].T

No max-subtraction in softmax: scores are O(10) so exp is safe in fp32; masked
entries give exp(-1e9+...) -> exactly 0, matching the reference's
exp(-1e9 - max) -> 0.
"""

import functools
import math

import numpy as np
import ml_dtypes

import concourse.bass as bass
import concourse.tile as tile
from concourse import bacc, mybir
from concourse.bass_utils import run_bass_kernel_spmd
from concourse.masks import make_identity

N = 3072
QD = 512
OD = 512
H = 8
D = 64
B = 8
NCORES = 8
SCALE = math.sqrt(D)

F32 = mybir.dt.float32
BF16 = mybir.dt.bfloat16
BF16_NP = ml_dtypes.bfloat16

TRACE = False
LAST_RESULTS = None


def _bounds(q_batch, k_batch):
    qb = np.asarray(q_batch).astype(np.int64)
    kb = np.asarray(k_batch).astype(np.int64)
    qbound = np.searchsorted(qb, np.arange(B + 1))
    kbound = np.searchsorted(kb, np.arange(B + 1))
    return qbound, kbound


def _chunks(lo, hi, step):
    return [(o, min(step, hi - o)) for o in range(lo, hi, step)]


def _r128(x):
    return max(128, ((x + 127) // 128) * 128)


@functools.lru_cache(maxsize=8)
def _build(NQ, NK, has_bq, has_bk, has_bv, has_bo):
    nc = bacc.Bacc("TRN2", target_bir_lowering=False, debug=False,
                   num_devices=NCORES)

    KT_T = QD // 128   # 4 contraction tiles for the projections
    NKC = NK // 128    # k chunks
    QCH = _chunks(0, NQ, 512)   # q chunks (free-dim <= 512)
    NTD = QD // 128    # output-d tiles for projections

    qfT_d = nc.dram_tensor("qfT", [QD, NQ], BF16, kind="ExternalInput")
    kfT_d = nc.dram_tensor("kfT", [QD, NK], BF16, kind="ExternalInput")
    vfT_d = nc.dram_tensor("vfT", [QD, NK], BF16, kind="ExternalInput")
    posc_d = nc.dram_tensor("posc", [H, NK, NQ], BF16, kind="ExternalInput")
    wq_d = nc.dram_tensor("wq", [QD, OD], BF16, kind="ExternalInput")
    wk_d = nc.dram_tensor("wk", [QD, OD], BF16, kind="ExternalInput")
    wv_d = nc.dram_tensor("wv", [QD, OD], BF16, kind="ExternalInput")
    wo_d = nc.dram_tensor("wo", [OD, OD], BF16, kind="ExternalInput")
    bq_d = nc.dram_tensor("bq", [1, OD], BF16, kind="ExternalInput") if has_bq else None
    bk_d = nc.dram_tensor("bk", [1, OD], BF16, kind="ExternalInput") if has_bk else None
    bv_d = nc.dram_tensor("bv", [1, OD], BF16, kind="ExternalInput") if has_bv else None
    bo_d = nc.dram_tensor("bo", [128, NTD], F32, kind="ExternalInput") if has_bo else None
    out_d = nc.dram_tensor("out", [OD, NQ], BF16, kind="ExternalOutput")
    import os
    DEBUG = bool(os.environ.get("KDBG"))
    if DEBUG:
        dbg_h = nc.dram_tensor("dbg_h", [D, H, NQ], F32, kind="ExternalOutput")
        dbg_z = nc.dram_tensor("dbg_z", [1, H, NQ], F32, kind="ExternalOutput")
        dbg_e = nc.dram_tensor("dbg_e", [128, NQ], F32, kind="ExternalOutput")

    with tile.TileContext(nc) as tc:
        with (
            tc.tile_pool(name="consts", bufs=1) as consts,
            tc.tile_pool(name="pos", bufs=12) as posp,
            tc.tile_pool(name="expp", bufs=12) as expp,
            tc.tile_pool(name="outp", bufs=4) as outp,
            tc.tile_pool(name="ps_s", bufs=4, space="PSUM") as ps_s,
            tc.tile_pool(name="ps_h", bufs=2, space="PSUM") as ps_h,
            tc.tile_pool(name="ps_p", bufs=2, space="PSUM") as ps_p,
            tc.tile_pool(name="dram", bufs=1, space="DRAM") as dramp,
        ):
            # ---------------- constants / weights ----------------
            ones = consts.tile([1, max(NQ, NK)], BF16)
            nc.vector.memset(ones, 1.0)
            ones_f = consts.tile([1, D], F32)
            nc.vector.memset(ones_f, 1.0)
            ident128 = consts.tile([128, 128], BF16)
            make_identity(nc, ident128)
            # HAM warmup: ~4us of dummy matmuls while the feature DMAs land,
            # so the projections start at 2.4 GHz instead of 1.2
            warm_ps = ps_p.tile([128, 512], F32, tag="psp")
            for wi in range(8):
                nc.tensor.matmul(warm_ps[:, 0:128], ident128[:, :],
                                 ident128[:, :], start=(wi == 0), stop=(wi == 7))
            warm_sb = consts.tile([1, 1], F32, name="warm_sb")
            nc.vector.tensor_copy(warm_sb[0:1, 0:1], warm_ps[0:1, 0:1])
            warm_d = dramp.tile([1, 1], F32)
            nc.gpsimd.dma_start(out=warm_d[:, :], in_=warm_sb[0:1, 0:1])

            wq_sb = consts.tile([128, KT_T, OD], BF16)
            wk_sb = consts.tile([128, KT_T, OD], BF16)
            wv_sb = consts.tile([128, KT_T, OD], BF16)
            pass
            # wo as [128, t, oc, 128] for contraction-128 output projection
            wo_sb = consts.tile([128, NTD, NTD, 128], BF16)
            nc.gpsimd.dma_start(
                out=wo_sb,
                in_=wo_d.ap().rearrange("(t p) (o c) -> p t o c", p=128, c=128))
            bias_sb = {}
            for nm, dd in (("bq", bq_d), ("bk", bk_d), ("bv", bv_d)):
                if dd is not None:
                    t = consts.tile([1, OD], BF16, tag=f"bias_{nm}", name=f"b_{nm}")
                    nc.gpsimd.dma_start(out=t, in_=dd[:, :])
                    bias_sb[nm] = t
            if bo_d is not None:
                bo_sb = consts.tile([128, NTD], F32)
                nc.gpsimd.dma_start(out=bo_sb, in_=bo_d[:, :])

            # feature tiles
            qf_sb = consts.tile([128, KT_T, NQ], BF16)
            kf_sb = consts.tile([128, KT_T, NK], BF16)
            vf_sb = consts.tile([128, KT_T, NK], BF16)
            for f_d, f_sb, w_d, w_sb in ((qfT_d, qf_sb, wq_d, wq_sb),
                                         (kfT_d, kf_sb, wk_d, wk_sb),
                                         (vfT_d, vf_sb, wv_d, wv_sb)):
                for t in range(KT_T):
                    nc.sync.dma_start(
                        out=f_sb[:, t, :],
                        in_=f_d.ap().rearrange("(t p) n -> t p n", p=128)[t])
                nc.sync.dma_start(
                    out=w_sb, in_=w_d.ap().rearrange("(t p) d -> p t d", p=128))

            # projected tensors, split by head parity so every matmul operand
            # sits at partition base 0 (slot index = head // 2)
            QT_f = consts.tile([128, NTD, NQ], BF16, name="QT_f")
            KT_f = consts.tile([128, NTD, NK], BF16, name="KT_f")
            VT_f = consts.tile([128, NTD, NK], BF16, name="VT_f")
            V_sb = consts.tile([128, NKC, H, D + 1], BF16, name="V_sb")
            hT_sb = consts.tile([D, H, NQ], BF16, name="hT_sb")
            hTn_sb = consts.tile([128, NTD, NQ], BF16, name="hTn_sb")
            zall_sb = consts.tile([1, H, NQ], F32, name="zall_sb")
            zrec_sb = consts.tile([1, H, NQ], F32, name="zrec_sb")

            # ---------------- projections ----------------
            kchunks = _chunks(0, NK, 512)

            def project_full(f_sb, w_sb, bias, dst, xchunks):
                for td in range(NTD):
                    dsl = slice(128 * td, 128 * (td + 1))
                    for (xo, xw) in xchunks:
                        xsl = slice(xo, xo + xw)
                        psum = ps_p.tile([128, 512], F32, tag="psp")
                        for t in range(KT_T):
                            nc.tensor.matmul(psum[:, 0:xw],
                                             w_sb[:, t, dsl], f_sb[:, t, xsl],
                                             start=(t == 0),
                                             stop=(t == KT_T - 1 and bias is None))
                        if bias is not None:
                            nc.tensor.matmul(psum[:, 0:xw], bias[:, dsl],
                                             ones[:, xsl], start=False, stop=True)
                        if td % 2 == 0:
                            nc.scalar.copy(dst[:, td, xsl], psum[:, 0:xw])
                        else:
                            nc.vector.tensor_copy(dst[:, td, xsl], psum[:, 0:xw])

            project_full(qf_sb, wq_sb, bias_sb.get("bq"), QT_f, QCH)
            project_full(kf_sb, wk_sb, bias_sb.get("bk"), KT_f, kchunks)
            project_full(vf_sb, wv_sb, bias_sb.get("bv"), VT_f, kchunks)

            # V into [k, d | ones] per (kchunk, d-tile) via full PE transposes;
            # each [128,128] transpose covers two heads' 64-dim halves
            nc.vector.memset(V_sb[:, :, :, D], 1.0)
            for kc in range(NKC):
                ksl = slice(128 * kc, 128 * (kc + 1))
                for td in range(NTD):
                    pst = ps_p.tile([128, 512], BF16, tag="psp")
                    nc.tensor.transpose(pst[:, 0:128], VT_f[:, td, ksl],
                                        ident128[:, :])
                    nc.scalar.copy(V_sb[:, kc, 2 * td, 0:D], pst[:, 0:D])
                    nc.vector.tensor_copy(V_sb[:, kc, 2 * td + 1, 0:D],
                                          pst[:, D:128])

            # ---------------- attention, software-pipelined ----------------
            units = [(h, qo, qw) for h in range(H) for (qo, qw) in QCH]
            expts = {}

            def stage1(i):
                h, qo, qw = units[i]
                qsl = slice(qo, qo + qw)
                po = D * (h % 2)
                psl = slice(po, po + D)
                lst = []
                for kc in range(NKC):
                    ksl = slice(128 * kc, 128 * (kc + 1))
                    ps = ps_s.tile([128, 512], F32, tag="pss")
                    nc.tensor.matmul(ps[:, 0:qw], KT_f[psl, h // 2, ksl],
                                     QT_f[psl, h // 2, qsl], start=True, stop=True)
                    pos = posp.tile([128, 512], BF16, tag="pos")
                    eng = nc.gpsimd if kc % 2 == 0 else nc.sync
                    eng.dma_start(out=pos[:, 0:qw], in_=posc_d[h, ksl, qsl])
                    expr = expp.tile([128, 512], BF16, tag="expr")
                    nc.scalar.activation(expr[:, 0:qw], ps[:, 0:qw],
                                         mybir.ActivationFunctionType.Exp)
                    # posc holds exp(pos): exp(s+p) = exp(s)*exp(p); both bf16
                    # SBUF operands so the multiply runs in the DVE 2x mode
                    expt = expp.tile([128, 512], BF16, tag="expt")
                    nc.vector.tensor_mul(expt[:, 0:qw], expr[:, 0:qw],
                                         pos[:, 0:qw])
                    lst.append(expt)
                expts[i] = lst

            def stage2(i):
                h, qo, qw = units[i]
                qsl = slice(qo, qo + qw)
                psum_h = ps_h.tile([D + 1, 512], F32, tag="psh")
                for kc in range(NKC):
                    nc.tensor.matmul(psum_h[:, 0:qw], V_sb[:, kc, h, :],
                                     expts[i][kc][:, 0:qw],
                                     start=(kc == 0), stop=(kc == NKC - 1))
                del expts[i]
                nc.scalar.copy(hT_sb[:, h, qsl], psum_h[0:D, 0:qw])
                # 1/Z for this unit (Z = row 64 of the accumulator); the
                # approx reciprocal requires an SBUF source, so copy first
                nc.scalar.copy(zall_sb[0:1, h, qsl], psum_h[D:D + 1, 0:qw])
                nc.vector.reciprocal_approx_fast(zrec_sb[0:1, h, qsl],
                                                 zall_sb[0:1, h, qsl])
                po = D * (h % 2)
                if i >= len(units) - 2:
                    # tail units: broadcast 1/Z with a K=1 matmul (PE is idle
                    # here; the DMA bounce would gate the output projection)
                    zb_ps = ps_p.tile([128, 512], F32, tag="psp")
                    nc.tensor.matmul(zb_ps[0:D, 0:qw], ones_f[0:1, :],
                                     zrec_sb[0:1, h, qsl], start=True, stop=True)
                    nc.vector.tensor_mul(hTn_sb[po:po + D, h // 2, qsl],
                                         hT_sb[:, h, qsl], zb_ps[0:D, 0:qw])
                else:
                    # broadcast 1/Z across the 64 d-partitions via a DRAM
                    # bounce + stride-0 partition read, then normalize hT
                    zr_d = dramp.tile([1, 512], F32, tag=f"zrd{i}", name=f"zrd{i}")
                    nc.gpsimd.dma_start(out=zr_d[0:1, 0:qw],
                                        in_=zrec_sb[0:1, h, qsl])
                    zr_ap = zr_d[:, :]
                    zbc_src = bass.AP(tensor=zr_ap.tensor, offset=zr_ap.offset,
                                      ap=[[0, D], [1, qw]])
                    zbc = posp.tile([D, 512], F32, tag="zbc", name="zbc")
                    nc.gpsimd.dma_start(out=zbc[:, 0:qw], in_=zbc_src)
                    nc.vector.tensor_mul(hTn_sb[po:po + D, h // 2, qsl],
                                         hT_sb[:, h, qsl], zbc[:, 0:qw])

            LOOK = 2
            for i in range(len(units)):
                stage1(i)
                if i >= LOOK:
                    stage2(i - LOOK)
            for i in range(max(0, len(units) - LOOK), len(units)):
                stage2(i)

            if DEBUG:
                dh = consts.tile([D, H, NQ], F32, name="dh")
                nc.vector.tensor_copy(dh[:, :, :], hT_sb[:, :, :])
                nc.gpsimd.dma_start(out=dbg_h.ap(), in_=dh[:, :, :])
                nc.gpsimd.dma_start(out=dbg_z.ap(), in_=zrec_sb[0:1, :, :])

            # ---------------- output projection ----------------
            for oc in range(NTD):
                for (qo, qw) in QCH:
                    qsl = slice(qo, qo + qw)
                    psum = ps_p.tile([128, 512], F32, tag="psp")
                    for t in range(NTD):
                        nc.tensor.matmul(psum[:, 0:qw], wo_sb[:, t, oc, :],
                                         hTn_sb[:, t, qsl],
                                         start=(t == 0), stop=(t == NTD - 1))
                    o_sb = outp.tile([128, 512], BF16, tag="osb")
                    if bo_d is not None:
                        nc.scalar.activation(o_sb[:, 0:qw], psum[:, 0:qw],
                                             mybir.ActivationFunctionType.Identity,
                                             bias=bo_sb[:, oc:oc + 1])
                    else:
                        nc.vector.tensor_copy(o_sb[:, 0:qw], psum[:, 0:qw])
                    nc.sync.dma_start(out=out_d[128 * oc:128 * (oc + 1), qsl],
                                      in_=o_sb[:, 0:qw])

    nc.compile()
    return nc


def _kernel_numpy(q_feat, k_feat, v_feat, pos_enc, Wq, bq, Wk, bk, Wv, bv,
                  Wo, bo, q_batch, k_batch):
    """Host fallback (degenerate batch layouts) + debugging aid."""
    Q = (q_feat @ Wq + bq).reshape(N, H, D).transpose(1, 0, 2)
    K = (k_feat @ Wk + bk).reshape(N, H, D).transpose(1, 0, 2)
    V = (v_feat @ Wv + bv).reshape(N, H, D).transpose(1, 0, 2)
    scores = np.einsum("hnd,hmd->hnm", Q, K) / SCALE + pos_enc
    mask = q_batch[:, None] != k_batch[None, :]
    scores = np.where(mask[None], np.float32(-1e9), scores)
    scores = scores - scores.max(-1, keepdims=True)
    e = np.exp(scores)
    probs = e / e.sum(-1, keepdims=True)
    h = np.einsum("hnm,hmd->hnd", probs, V)
    h = h.transpose(1, 0, 2).reshape(N, OD)
    return (h @ Wo + bo).astype(np.float32)


def kernel(q_feat, k_feat, v_feat, pos_enc, Wq, bq, Wk, bk, Wv, bv, Wo, bo,
           q_batch, k_batch):
    global LAST_RESULTS
    args = dict(q_feat=np.asarray(q_feat, np.float32),
                k_feat=np.asarray(k_feat, np.float32),
                v_feat=np.asarray(v_feat, np.float32),
                pos_enc=np.asarray(pos_enc, np.float32),
                Wq=np.asarray(Wq, np.float32), bq=np.asarray(bq, np.float32),
                Wk=np.asarray(Wk, np.float32), bk=np.asarray(bk, np.float32),
                Wv=np.asarray(Wv, np.float32), bv=np.asarray(bv, np.float32),
                Wo=np.asarray(Wo, np.float32), bo=np.asarray(bo, np.float32),
                q_batch=np.asarray(q_batch), k_batch=np.asarray(k_batch))

    qbound, kbound = _bounds(args["q_batch"], args["k_batch"])
    nq_all = np.diff(qbound)
    nk_all = np.diff(kbound)
    if np.any((nq_all > 0) & (nk_all == 0)):
        # some batch has queries but no keys: reference gives uniform attention
        # over ALL keys there; fall back (never happens with realistic inputs)
        return _kernel_numpy(**args)

    NQ = _r128(int(nq_all.max()))
    NK = _r128(int(nk_all.max()))

    has_bq = bool(np.any(args["bq"]))
    has_bk = bool(np.any(args["bk"]))
    has_bv = bool(np.any(args["bv"]))
    has_bo = bool(np.any(args["bo"]))

    nc = _build(NQ, NK, has_bq, has_bk, has_bv, has_bo)

    # ---- host-side sharding / layout / padding ----
    qfT = np.ascontiguousarray(args["q_feat"].T).astype(BF16_NP)
    kfT = np.ascontiguousarray(args["k_feat"].T).astype(BF16_NP)
    vfT = np.ascontiguousarray(args["v_feat"].T).astype(BF16_NP)
    wq8 = (args["Wq"] / SCALE).astype(BF16_NP)
    wkb = args["Wk"].astype(BF16_NP)
    wvb = args["Wv"].astype(BF16_NP)
    wob = np.ascontiguousarray(args["Wo"]).astype(BF16_NP)

    in_maps = []
    for c in range(NCORES):
        qs, qe = int(qbound[c]), int(qbound[c + 1])
        ks, ke = int(kbound[c]), int(kbound[c + 1])
        nq, nk = qe - qs, ke - ks

        qfc = np.zeros((QD, NQ), BF16_NP)
        qfc[:, :nq] = qfT[:, qs:qe]
        kfc = np.zeros((QD, NK), BF16_NP)
        kfc[:, :nk] = kfT[:, ks:ke]
        vfc = np.zeros((QD, NK), BF16_NP)
        vfc[:, :nk] = vfT[:, ks:ke]

        # posc holds exp(pos): 0 on masked pad-k rows, 1 on pad-q cols
        posc = np.zeros((H, NK, NQ), BF16_NP)
        if nk > 0:
            posc[:, :nk, :] = 1.0
            posc[:, :nk, :nq] = np.exp(args["pos_enc"][:, qs:qe, ks:ke]) \
                .swapaxes(1, 2).astype(BF16_NP)

        m = {"qfT": qfc, "kfT": kfc, "vfT": vfc, "posc": posc,
             "wq": wq8, "wk": wkb, "wv": wvb, "wo": wob}
        if has_bq:
            m["bq"] = (args["bq"] / SCALE).astype(BF16_NP).reshape(1, OD)
        if has_bk:
            m["bk"] = args["bk"].astype(BF16_NP).reshape(1, OD)
        if has_bv:
            m["bv"] = args["bv"].astype(BF16_NP).reshape(1, OD)
        if has_bo:
            m["bo"] = np.ascontiguousarray(
                args["bo"].astype(np.float32).reshape(OD // 128, 128).T)
        in_maps.append(m)

    res = run_bass_kernel_spmd(nc, in_maps, core_ids=list(range(NCORES)),
                               trace=TRACE)
    LAST_RESULTS = res
    out = np.empty((N, OD), np.float32)
    for c in range(NCORES):
        qs, qe = int(qbound[c]), int(qbound[c + 1])
        if qe > qs:
            out[qs:qe, :] = res.results[c]["out"][:, :qe - qs].T.astype(np.float32)
    return out



# revision 28
# speedup vs baseline: 1.0671x; 1.0671x over previous
"""Trainium2 Bass kernel for nn_MultiHeadAttention_3796751090171 (sparse_attention).

Batch-parallel SPMD across 8 NeuronCores: q_batch/k_batch are SORTED, so the
cross-batch mask makes attention block-diagonal over batches, and there are
exactly B=8 batches for 8 cores. Core c computes batch c's queries against
batch c's keys for ALL 8 heads -- completely independent work, NO collectives.

v2 redesign vs the 81us baseline (trace-driven):
  - exact shapes: NQ = max batch q-count (416), k padded to 128-multiple
    (512) -- matmul cost on PE is output-columns x 1, so trimming q from
    512 -> 416 cuts every attention matmul, exp, mult and posc DMA by ~19%.
  - V projected DIRECTLY in [k, d] layout (features as stationary), killing
    the 16 PE transposes + extra copies of the baseline.
  - scores for two k-chunks land in ONE bf16 PSUM pair-tile [128, 2*NQ], so
    exp runs 16x [128,832] ACT ops instead of 32x [128,512] (ACT cost is
    free-size + ~180ns/op overhead).
  - output projection is RUNNING: after each head's AV + normalize, 4
    K=64 matmuls accumulate into 4 persistent PSUM banks -- no serial
    oproj tail, no hTn staging buffer.
  - 1/Z: Z row (from the ones-column of the AV matmul) is copied with hT
    to SBUF, broadcast across 64 partitions with a K=1 matmul into a
    rotating PSUM tile, and applied with a single DVE divide (no DRAM
    bounce, no reciprocal).
  - DMA: big per-tensor transfers, K-proj inputs chunked first so the PE
    starts ~1.5us in; posc streamed per-head (1 DMA each) on gpsimd.
  - warmup matmul burst from t=0 keeps the HAM power-state ramp going so
    projections hit 2.4GHz earlier.

PSUM budget (8 banks): 2 (score pairs / K proj / warmup) + 2 (Q,V proj /
AV accum / Z broadcast) + 4 (running oproj accumulators).
"""

import functools
import math

import numpy as np
import ml_dtypes

import concourse.bass as bass
import concourse.tile as tile
from concourse import bacc, mybir
from concourse.bass_utils import run_bass_kernel_spmd

N = 3072
QD = 512
OD = 512
H = 8
D = 64
B = 8
NCORES = 8
SCALE = math.sqrt(D)

F32 = mybir.dt.float32
BF16 = mybir.dt.bfloat16
BF16_NP = ml_dtypes.bfloat16

TRACE = False
LAST_RESULTS = None

# schedule tuning knobs
WARM = 24          # warmup matmuls (128 cols each) while first DMAs land
LOOK_B = 1         # slots between score pair and its exp+mult
LOOK_C = 2         # slots between score pair and its AV pair
LAG_D2 = 1         # extra slots before a head's oproj matmuls


def _bounds(q_batch, k_batch):
    qb = np.asarray(q_batch).astype(np.int64)
    kb = np.asarray(k_batch).astype(np.int64)
    qbound = np.searchsorted(qb, np.arange(B + 1))
    kbound = np.searchsorted(kb, np.arange(B + 1))
    return qbound, kbound


@functools.lru_cache(maxsize=8)
def _build(NQ, NKP, has_bq, has_bk, has_bv, has_bo):
    nc = bacc.Bacc("TRN2", target_bir_lowering=False, debug=False,
                   num_devices=NCORES)

    KT = QD // 128     # 4 feature-contraction chunks
    NKC = NKP // 128   # k chunks
    NTD = OD // 128    # output-dim tiles for Q/K projections

    qfT_d = nc.dram_tensor("qfT", [QD, NQ], BF16, kind="ExternalInput")
    kfT_d = nc.dram_tensor("kfT", [QD, NKP], BF16, kind="ExternalInput")
    vfT_d = nc.dram_tensor("vfT", [QD, NKP], BF16, kind="ExternalInput")
    posc_d = nc.dram_tensor("posc", [H, NKP, NQ], BF16, kind="ExternalInput")
    wq_d = nc.dram_tensor("wq", [QD, OD], BF16, kind="ExternalInput")
    wk_d = nc.dram_tensor("wk", [QD, OD], BF16, kind="ExternalInput")
    wv_d = nc.dram_tensor("wv", [QD, OD], BF16, kind="ExternalInput")
    woh_d = nc.dram_tensor("woh", [D, H * OD], BF16, kind="ExternalInput")
    bq_d = nc.dram_tensor("bq", [128, NTD], F32, kind="ExternalInput") if has_bq else None
    bk_d = nc.dram_tensor("bk", [128, NTD], F32, kind="ExternalInput") if has_bk else None
    bv_d = nc.dram_tensor("bv", [1, OD], BF16, kind="ExternalInput") if has_bv else None
    bo_d = nc.dram_tensor("bo", [128, NTD], F32, kind="ExternalInput") if has_bo else None
    out_d = nc.dram_tensor("out", [OD, NQ], BF16, kind="ExternalOutput")
    import os
    DEBUG = bool(os.environ.get("KDBG"))
    if DEBUG:
        dbg_q = nc.dram_tensor("dbg_q", [128, NTD, NQ], BF16, kind="ExternalOutput")
        dbg_k = nc.dram_tensor("dbg_k", [128, NTD, NKP], BF16, kind="ExternalOutput")
        dbg_v = nc.dram_tensor("dbg_v", [128, NKC, H, D + 1], BF16, kind="ExternalOutput")
        dbg_ht = nc.dram_tensor("dbg_ht", [H, D + 1, NQ], F32, kind="ExternalOutput")
        dbg_em = nc.dram_tensor("dbg_em", [4, 128, 2 * NQ], BF16, kind="ExternalOutput")
        dbg_hn = nc.dram_tensor("dbg_hn", [H, D, NQ], BF16, kind="ExternalOutput")
        dbg_zr = nc.dram_tensor("dbg_zr", [H, NQ], F32, kind="ExternalOutput")
        dbg_zrb = nc.dram_tensor("dbg_zrb", [H, NQ], BF16, kind="ExternalOutput")

    with tile.TileContext(nc) as tc:
        with (
            tc.tile_pool(name="consts", bufs=1) as consts,
            tc.tile_pool(name="posp", bufs=3) as posp,
            tc.tile_pool(name="expp", bufs=3) as expp,
            tc.tile_pool(name="hp", bufs=2) as hp,
            tc.tile_pool(name="hnp", bufs=2) as hnp,
            tc.tile_pool(name="outp", bufs=4) as outp,
            tc.tile_pool(name="ps_s", bufs=2, space="PSUM") as ps_s,
            tc.tile_pool(name="ps_av", bufs=2, space="PSUM") as ps_av,
            tc.tile_pool(name="ps_o", bufs=1, space="PSUM") as ps_o,
            tc.tile_pool(name="dram", bufs=1, space="DRAM") as dramp,
        ):
            # ---------------- warmup: keep the PE busy from t=0 so the HAM
            # activity ramp (1.2 -> 2.4 GHz) starts before the projections.
            wtile = consts.tile([128, 128], BF16, name="wtile")
            nc.vector.memset(wtile, 0.5)
            # row 64 of a [65, D] ones tile: partition base matches the Z row
            # of the AV accumulator for the K=1 broadcast matmul
            ones64 = consts.tile([D + 1, D], BF16, name="ones64")
            nc.vector.memset(ones64, 1.0)
            warm_ps = ps_s.tile([128, 512], F32, tag="pss", name="warm_ps")
            for wi in range(WARM):
                nc.tensor.matmul(warm_ps[:, 0:128], wtile[:, :], wtile[:, :],
                                 start=(wi % 8 == 0),
                                 stop=(wi % 8 == 7 or wi == WARM - 1))
            warm_sb = consts.tile([1, 1], F32, name="warm_sb")
            nc.vector.tensor_copy(warm_sb[0:1, 0:1], warm_ps[0:1, 0:1])
            warm_d = dramp.tile([1, 1], F32, name="warm_d")
            nc.gpsimd.dma_start(out=warm_d[:, :], in_=warm_sb[0:1, 0:1])

            # ---------------- weights / features (DMA issue order matters:
            # K-proj inputs first, finely chunked, so the PE starts early)
            wk_sb = consts.tile([128, KT, OD], BF16, name="wk_sb")
            kf_sb = consts.tile([128, KT, NKP], BF16, name="kf_sb")
            wq_sb = consts.tile([128, KT, OD], BF16, name="wq_sb")
            qf_sb = consts.tile([128, KT, NQ], BF16, name="qf_sb")
            wv_sb = consts.tile([128, KT, OD], BF16, name="wv_sb")
            vf_sb = consts.tile([128, KT, NKP], BF16, name="vf_sb")

            wk_ap = wk_d.ap().rearrange("(t p) d -> t p d", p=128)
            kf_ap = kfT_d.ap().rearrange("(t p) n -> t p n", p=128)
            for t in range(KT):
                nc.sync.dma_start(out=wk_sb[:, t, :], in_=wk_ap[t])
                nc.sync.dma_start(out=kf_sb[:, t, :], in_=kf_ap[t])
            nc.sync.dma_start(
                out=wq_sb, in_=wq_d.ap().rearrange("(t p) d -> p t d", p=128))
            nc.sync.dma_start(
                out=qf_sb, in_=qfT_d.ap().rearrange("(t p) n -> p t n", p=128))
            nc.sync.dma_start(
                out=wv_sb, in_=wv_d.ap().rearrange("(t p) d -> p t d", p=128))
            nc.sync.dma_start(
                out=vf_sb, in_=vfT_d.ap().rearrange("(t p) n -> p t n", p=128))

            wo_sb = consts.tile([D, H, OD], BF16, name="wo_sb")
            nc.gpsimd.dma_start(
                out=wo_sb, in_=woh_d.ap().rearrange("d (h o) -> d h o", h=H))

            bias_sb = {}
            for nm, dd in (("bq", bq_d), ("bk", bk_d), ("bo", bo_d)):
                if dd is not None:
                    t = consts.tile([128, NTD], F32, tag=f"b_{nm}", name=f"b_{nm}")
                    nc.gpsimd.dma_start(out=t, in_=dd[:, :])
                    bias_sb[nm] = t
            if bv_d is not None:
                bv_sb = consts.tile([1, OD], BF16, name="bv_sb")
                nc.gpsimd.dma_start(out=bv_sb, in_=bv_d[:, :])
                ones1 = consts.tile([1, 128], BF16, name="ones1")
                nc.vector.memset(ones1, 1.0)

            # projected tensors
            KT_f = consts.tile([128, NTD, NKP], BF16, name="KT_f")
            QT_f = consts.tile([128, NTD, NQ], BF16, name="QT_f")
            V_sb = consts.tile([128, NKC, H, D + 1], BF16, name="V_sb")
            # ones column for the fused-Z row of the AV matmul
            nc.vector.memset(V_sb[:, :, :, D], 1.0)

            # ---------------- K / Q projections (out = W^T X, d on partitions)
            def proj(f_sb, w_sb, dst, ncols, bias, pool, ptag):
                for td in range(NTD):
                    dsl = slice(128 * td, 128 * (td + 1))
                    ps = pool.tile([128, 512], F32, tag=ptag, name="projps")
                    for t in range(KT):
                        nc.tensor.matmul(ps[:, 0:ncols], w_sb[:, t, dsl],
                                         f_sb[:, t, 0:ncols],
                                         start=(t == 0), stop=(t == KT - 1))
                    if bias is not None:
                        nc.scalar.activation(dst[:, td, 0:ncols], ps[:, 0:ncols],
                                             mybir.ActivationFunctionType.Identity,
                                             bias=bias[:, td:td + 1])
                    elif td % 2 == 0:
                        nc.scalar.copy(dst[:, td, 0:ncols], ps[:, 0:ncols])
                    else:
                        nc.vector.tensor_copy(dst[:, td, 0:ncols], ps[:, 0:ncols])

            proj(kf_sb, wk_sb, KT_f, NKP, bias_sb.get("bk"), ps_s, "pss")
            proj(qf_sb, wq_sb, QT_f, NQ, bias_sb.get("bq"), ps_av, "avt")

            # ---------------- V projection, direct [k, d] layout:
            # out[k, d] += vf_chunk^T @ wv_chunk  (features stationary)
            for c in range(NKC):
                ksl = slice(128 * c, 128 * (c + 1))
                ps = ps_av.tile([128, 512], F32, tag="avt", name="vps")
                for t in range(KT):
                    nc.tensor.matmul(ps[:, 0:OD], vf_sb[:, t, ksl],
                                     wv_sb[:, t, :],
                                     start=(t == 0),
                                     stop=(t == KT - 1 and bv_d is None))
                if bv_d is not None:
                    nc.tensor.matmul(ps[:, 0:OD], ones1[:, 0:128],
                                     bv_sb[:, :], start=False, stop=True)
                src = ps[:, 0:OD].rearrange("p (h d) -> p h d", h=H)
                if c % 2 == 0:
                    nc.scalar.copy(V_sb[:, c, :, 0:D], src)
                else:
                    nc.vector.tensor_copy(V_sb[:, c, :, 0:D], src)

            # ---------------- attention: slots = (head, kchunk-pair) --------
            NP = NKC // 2          # kchunk pairs per head (2)
            S = H * NP             # 16 slots
            st_ps = {}
            expm = {}
            avps = {}
            hts = {}
            pos_tiles = {}
            posc_ap = posc_d.ap().rearrange("h (c p) q -> h p c q", p=128)

            def issue_pos(h):
                t = posp.tile([128, NKC, NQ], BF16, tag="pos", name="pos")
                nc.gpsimd.dma_start(out=t, in_=posc_ap[h])
                pos_tiles[h] = t

            for h in range(min(3, H)):
                issue_pos(h)

            def stageA(s):  # score pair matmuls (two f32 psum tiles)
                h, p = s // NP, s % NP
                if p == 0 and h + 3 < H:
                    issue_pos(h + 3)
                po = D * (h % 2)
                tiles = []
                for ci in (0, 1):
                    c = 2 * p + ci
                    ksl = slice(128 * c, 128 * (c + 1))
                    pst = ps_s.tile([128, NQ], F32, tag="pss", name="pst")
                    nc.tensor.matmul(pst[:, 0:NQ],
                                     KT_f[po:po + D, h // 2, ksl],
                                     QT_f[po:po + D, h // 2, 0:NQ],
                                     start=True, stop=True)
                    tiles.append(pst)
                st_ps[s] = tiles

            def stageD1(h):  # hT+Z to SBUF, 1/Z (gpsimd pow), Z broadcast
                ht = hp.tile([D + 1, NQ], F32, tag="hT", name="ht")
                if h % 2 == 0:
                    nc.scalar.copy(ht[:, :], avps[h][:, 0:NQ])
                else:
                    nc.vector.tensor_copy(ht[:, :], avps[h][:, 0:NQ])
                del avps[h]
                # full-tile recip: the custom DVE uop mishandles partition-
                # base-64 single-row APs; lanes are parallel so [65,NQ] costs
                # the same and row 64 gives 1/Z (rows 0..63 are junk, unread)
                zr = hp.tile([D + 1, NQ], F32, tag="zr", name="zr")
                nc.vector.reciprocal_approx_fast(zr[:, :], ht[:, :])
                zrb = hp.tile([D + 1, NQ], BF16, tag="zrb", name="zrb")
                nc.scalar.copy(zrb[D:D + 1, :], zr[D:D + 1, :])
                zbc = ps_av.tile([D, NQ], F32, tag="avt", name="zbc")
                nc.tensor.matmul(zbc[:, 0:NQ], ones64[D:D + 1, :],
                                 zrb[D:D + 1, :], start=True, stop=True)
                if DEBUG:
                    nc.sync.dma_start(out=dbg_ht.ap()[h], in_=ht[:, :])
                    nc.sync.dma_start(out=dbg_zr.ap()[h:h + 1, :],
                                      in_=zr[D:D + 1, :])
                    nc.sync.dma_start(out=dbg_zrb.ap()[h:h + 1, :],
                                      in_=zrb[D:D + 1, :])
                hts[h] = (ht, zbc)

            def stageB(s):  # exp x2 (ACT) + paired posc multiply (DVE)
                h, p = s // NP, s % NP
                ex = expp.tile([128, 2 * NQ], BF16, tag="expr", name="ex")
                for ci in (0, 1):
                    nc.scalar.activation(ex[:, NQ * ci:NQ * (ci + 1)],
                                         st_ps[s][ci][:, 0:NQ],
                                         mybir.ActivationFunctionType.Exp)
                del st_ps[s]
                em = expp.tile([128, 2 * NQ], BF16, tag="expm", name="em")
                nc.vector.tensor_mul(
                    em[:, :].rearrange("p (c q) -> p c q", c=2),
                    ex[:, :].rearrange("p (c q) -> p c q", c=2),
                    pos_tiles[h][:, 2 * p:2 * p + 2, :])
                if DEBUG and s < 4:
                    nc.sync.dma_start(out=dbg_em.ap()[s], in_=em[:, :])
                expm[s] = em

            def stageC(s):  # AV pair (accumulate [hT | Z] per head)
                h, p = s // NP, s % NP
                if p == 0:
                    avps[h] = ps_av.tile([D + 1, NQ], F32, tag="avt", name="avt")
                for ci in (0, 1):
                    c = 2 * p + ci
                    nc.tensor.matmul(avps[h][:, 0:NQ], V_sb[:, c, h, :],
                                     expm[s][:, NQ * ci:NQ * (ci + 1)],
                                     start=(c == 0), stop=(c == NKC - 1))
                del expm[s]

            def stageD2(h):  # normalize + running output projection
                ht, zbc = hts.pop(h)
                hn = hnp.tile([D, NQ], BF16, tag="hTn", name="hn")
                nc.vector.tensor_mul(hn[:, :], ht[0:D, :], zbc[:, 0:NQ])
                if DEBUG:
                    nc.sync.dma_start(out=dbg_hn.ap()[h], in_=hn[:, :])
                for oc in range(NTD):
                    nc.tensor.matmul(ps_o_t[oc][:, 0:NQ],
                                     wo_sb[:, h, 128 * oc:128 * (oc + 1)],
                                     hn[:, :],
                                     start=(h == 0), stop=(h == H - 1),
                                     skip_group_check=True)

            ps_o_t = [ps_o.tile([128, NQ], F32, tag=f"o{oc}", name=f"po{oc}")
                      for oc in range(NTD)]

            # D1(h) fires in the same slot as its p1 AV pair (right after it);
            # D2(h) LAG_D2 slots later.
            for s in range(S + 2 + LAG_D2 + 1):
                if s < S:
                    stageA(s)
                sd = s - LOOK_C  # slot whose AV-pair was just issued
                if 0 <= s - LOOK_B < S:
                    stageB(s - LOOK_B)
                if 0 <= sd < S:
                    stageC(sd)
                    if sd % NP == NP - 1:
                        stageD1(sd // NP)
                sd2 = s - LOOK_C - LAG_D2
                if 0 <= sd2 < S and sd2 % NP == NP - 1:
                    stageD2(sd2 // NP)

            if DEBUG:
                nc.sync.dma_start(out=dbg_q.ap(), in_=QT_f[:, :, :])
                nc.sync.dma_start(out=dbg_k.ap(), in_=KT_f[:, :, :])
                nc.sync.dma_start(out=dbg_v.ap(), in_=V_sb[:, :, :, :])

            # ---------------- final stores ----------------
            for oc in range(NTD):
                osb = outp.tile([128, NQ], BF16, tag="osb", name="osb")
                if bo_d is not None:
                    nc.scalar.activation(osb[:, :], ps_o_t[oc][:, 0:NQ],
                                         mybir.ActivationFunctionType.Identity,
                                         bias=bias_sb["bo"][:, oc:oc + 1])
                elif oc % 2 == 0:
                    nc.scalar.copy(osb[:, :], ps_o_t[oc][:, 0:NQ])
                else:
                    nc.vector.tensor_copy(osb[:, :], ps_o_t[oc][:, 0:NQ])
                nc.sync.dma_start(out=out_d[128 * oc:128 * (oc + 1), 0:NQ],
                                  in_=osb[:, :])

    nc.compile()
    return nc


def _kernel_numpy(q_feat, k_feat, v_feat, pos_enc, Wq, bq, Wk, bk, Wv, bv,
                  Wo, bo, q_batch, k_batch):
    """Host fallback (degenerate batch layouts)."""
    Q = (q_feat @ Wq + bq).reshape(N, H, D).transpose(1, 0, 2)
    K = (k_feat @ Wk + bk).reshape(N, H, D).transpose(1, 0, 2)
    V = (v_feat @ Wv + bv).reshape(N, H, D).transpose(1, 0, 2)
    scores = np.einsum("hnd,hmd->hnm", Q, K) / SCALE + pos_enc
    mask = q_batch[:, None] != k_batch[None, :]
    scores = np.where(mask[None], np.float32(-1e9), scores)
    scores = scores - scores.max(-1, keepdims=True)
    e = np.exp(scores)
    probs = e / e.sum(-1, keepdims=True)
    h = np.einsum("hnm,hmd->hnd", probs, V)
    h = h.transpose(1, 0, 2).reshape(N, OD)
    return (h @ Wo + bo).astype(np.float32)


def kernel(q_feat, k_feat, v_feat, pos_enc, Wq, bq, Wk, bk, Wv, bv, Wo, bo,
           q_batch, k_batch):
    global LAST_RESULTS
    args = dict(q_feat=np.asarray(q_feat, np.float32),
                k_feat=np.asarray(k_feat, np.float32),
                v_feat=np.asarray(v_feat, np.float32),
                pos_enc=np.asarray(pos_enc, np.float32),
                Wq=np.asarray(Wq, np.float32), bq=np.asarray(bq, np.float32),
                Wk=np.asarray(Wk, np.float32), bk=np.asarray(bk, np.float32),
                Wv=np.asarray(Wv, np.float32), bv=np.asarray(bv, np.float32),
                Wo=np.asarray(Wo, np.float32), bo=np.asarray(bo, np.float32),
                q_batch=np.asarray(q_batch), k_batch=np.asarray(k_batch))

    qbound, kbound = _bounds(args["q_batch"], args["k_batch"])
    nq_all = np.diff(qbound)
    nk_all = np.diff(kbound)
    if np.any((nq_all > 0) & (nk_all == 0)) or nq_all.max() == 0:
        # a batch with queries but no keys gets uniform attention over ALL
        # keys in the reference; fall back (never happens for real inputs)
        return _kernel_numpy(**args)

    NQ = int(nq_all.max())
    NKP = max(128, ((int(nk_all.max()) + 127) // 128) * 128)
    if NQ > 512 or NKP > 512:
        return _kernel_numpy(**args)

    has_bq = bool(np.any(args["bq"]))
    has_bk = bool(np.any(args["bk"]))
    has_bv = bool(np.any(args["bv"]))
    has_bo = bool(np.any(args["bo"]))

    nc = _build(NQ, NKP, has_bq, has_bk, has_bv, has_bo)

    NKC = NKP // 128
    NTD = OD // 128

    # ---- host-side sharding / layout / padding ----
    qfT = np.ascontiguousarray(args["q_feat"].T).astype(BF16_NP)
    kfT = np.ascontiguousarray(args["k_feat"].T).astype(BF16_NP)
    vfT = np.ascontiguousarray(args["v_feat"].T).astype(BF16_NP)
    wq8 = (args["Wq"] / SCALE).astype(BF16_NP)
    wkb = args["Wk"].astype(BF16_NP)
    wvb = args["Wv"].astype(BF16_NP)
    woh = np.ascontiguousarray(
        args["Wo"].reshape(H, D, OD).transpose(1, 0, 2).reshape(D, H * OD)
    ).astype(BF16_NP)

    in_maps = []
    for c in range(NCORES):
        qs, qe = int(qbound[c]), int(qbound[c + 1])
        ks, ke = int(kbound[c]), int(kbound[c + 1])
        nq, nk = qe - qs, ke - ks

        qfc = np.zeros((QD, NQ), BF16_NP)
        qfc[:, :nq] = qfT[:, qs:qe]
        kfc = np.zeros((QD, NKP), BF16_NP)
        kfc[:, :nk] = kfT[:, ks:ke]
        vfc = np.zeros((QD, NKP), BF16_NP)
        vfc[:, :nk] = vfT[:, ks:ke]

        # posc holds exp(pos): 0 on masked/pad k rows, 1 on pad-q columns
        posc = np.zeros((H, NKP, NQ), BF16_NP)
        if nk > 0:
            posc[:, :nk, :] = 1.0
            posc[:, :nk, :nq] = np.exp(
                args["pos_enc"][:, qs:qe, ks:ke]).swapaxes(1, 2).astype(BF16_NP)

        m = {"qfT": qfc, "kfT": kfc, "vfT": vfc, "posc": posc,
             "wq": wq8, "wk": wkb, "wv": wvb, "woh": woh}
        if has_bq:
            m["bq"] = np.ascontiguousarray(
                (args["bq"] / SCALE).astype(np.float32).reshape(NTD, 128).T)
        if has_bk:
            m["bk"] = np.ascontiguousarray(
                args["bk"].astype(np.float32).reshape(NTD, 128).T)
        if has_bv:
            m["bv"] = args["bv"].astype(BF16_NP).reshape(1, OD)
        if has_bo:
            m["bo"] = np.ascontiguousarray(
                args["bo"].astype(np.float32).reshape(NTD, 128).T)
        in_maps.append(m)

    res = run_bass_kernel_spmd(nc, in_maps, core_ids=list(range(NCORES)),
                               trace=TRACE)
    LAST_RESULTS = res
    out = np.empty((N, OD), np.float32)
    for c in range(NCORES):
        qs, qe = int(qbound[c]), int(qbound[c + 1])
        if qe > qs:
            out[qs:qe, :] = res.results[c]["out"][:, :qe - qs].T.astype(np.float32)
    return out
